# revision 54
# baseline (speedup 1.0000x reference)
"""Trainium2 Bass kernel for nn_CapsuleLayer (B=64, L=512, D=1024, C=32, O=64).

u-hat-free formulation. The reference computes u_hat = x @ fc_w (B*L*D*CO
MACs -- 218us/core of PE at bf16) and then routes. But routing only ever
consumes u_hat through two contractions:

    s_j[c,o]   = sum_l c_ij[l,c] * u[l,c,o] = ((c_ij^T @ x) @ fc_w)[c, (c,o)]
    delta[l,c] = sum_o u[l,c,o] * v[c,o]   = (x @ (fc_w . v))[l, c]

so u_hat never needs to be materialized.  Per iteration and batch element:
    y    = c^T x                  (C x D;   PE, 4-way col-tiled)
    s    = diag(y @ w) + colsum(c)*bias     (PE cross + DVE extract)
    v    = squash(s)
    Wv   = sum_o w[d,(c,o)] v[c,o]          (PE via block-diag vsel weights)
    dT   = Wv^T @ x^T + bcorr               (PE, 4-way col-tiled)
    b   += dT^T ; c = softmax(b)
All matmuls keep M=128 via tile_position col-tiling of 4 batch elements.
Data-parallel over batch: 8 cores x 8 batch elements (2 groups of 4).
"""

import contextlib
import ctypes
import sys
import types

import numpy as np
import ml_dtypes

DBG = False
DBG_IT = 0

B, L, D = 64, 512, 1024
C, O = 32, 64
CO = C * O
ITERS = 3
NCORES = 8
BPC = B // NCORES           # 8 batch elements per core
GI = 4                      # batch elements per group (col-tiling width)
NGRP = BPC // GI            # 2
P = 128
LT = L // P                 # 4 l-chunks
KD = D // P                 # 8 d-chunks
DB = D // 512               # 2 d-banks
MT = CO // P                # 16 co-chunks
NB = CO // 512              # 4 co-banks

_BF16 = ml_dtypes.bfloat16

# ---------------------------------------------------------------------------
# NTFF profiling shim (used when tracing is requested by the test harness)
# ---------------------------------------------------------------------------


def _install_ntff_shim():
    if "antenv.axon_hooks" in sys.modules:
        return
    so_path = "/opt/axon/libaxon_pjrt.so"
    hook = None
    try:
        lib = ctypes.CDLL(so_path)
        if hasattr(lib, "axon_start_nrt_profile"):
            lib.axon_start_nrt_profile.argtypes = [
                ctypes.POINTER(ctypes.c_int64),
                ctypes.c_size_t,
            ]
            lib.axon_start_nrt_profile.restype = ctypes.c_int64
            lib.axon_stop_nrt_profile.argtypes = [ctypes.c_char_p]
            lib.axon_stop_nrt_profile.restype = ctypes.c_int64

            @contextlib.contextmanager
            def hook(output_dir, device_ids):
                import jax

                jax.devices()
                if device_ids:
                    ids = (ctypes.c_int64 * len(device_ids))(*device_ids)
                    rc = lib.axon_start_nrt_profile(ids, len(device_ids))
                else:
                    rc = lib.axon_start_nrt_profile(None, 0)
                if rc != 0:
                    raise RuntimeError(f"axon_start_nrt_profile rc={rc}")
                try:
                    yield
                finally:
                    n = lib.axon_stop_nrt_profile(str(output_dir).encode())
                    if n < 0:
                        raise RuntimeError(f"axon_stop_nrt_profile rc={n}")
    except OSError:
        pass
    mod = types.ModuleType("antenv.axon_hooks")
    mod.get_axon_ntff_profile_hook = lambda: hook
    mod.set_axon_ntff_profile_hook = lambda h: None
    sys.modules["antenv.axon_hooks"] = mod

    import concourse.bass_utils as bu

    bu.upload_artifacts = lambda tmpdir: tmpdir


# ---------------------------------------------------------------------------
# Kernel builder
# ---------------------------------------------------------------------------


def build_kernel():
    import concourse.bacc as bacc
    import concourse.tile as tile
    import concourse.mybir as mybir

    f32 = mybir.dt.float32
    bf16 = mybir.dt.bfloat16
    AF = mybir.ActivationFunctionType
    ALU = mybir.AluOpType
    AX = mybir.AxisListType

    nc = bacc.Bacc("TRN2", target_bir_lowering=False, debug=False)

    dbg_d = {}
    if DBG:
        def _dbg(name, shape, dt):
            dbg_d[name] = nc.dram_tensor(name, shape, dt, kind="ExternalOutput")
        _dbg("dbg_y0", [P, D], bf16)
        _dbg("dbg_yt0", [P, KD, P], bf16)
        _dbg("dbg_s0", [P, O], f32)
        _dbg("dbg_v0", [P, O], f32)
        _dbg("dbg_vt0", [O, P], bf16)
        _dbg("dbg_vf0", [P, MT], bf16)
        _dbg("dbg_vs0", [P, MT, C], bf16)
        _dbg("dbg_wv0", [P, D], bf16)
        _dbg("dbg_wvt0", [P, KD, P], bf16)
        _dbg("dbg_ds0", [P, 512], bf16)
        _dbg("dbg_b0", [P, LT, P], f32)
        _dbg("dbg_c1", [P, LT, GI, C], bf16)
        _dbg("dbg_cs", [P, 1], f32)

    x_d = nc.dram_tensor("x", [BPC, P, LT, D], bf16, kind="ExternalInput")
    xt_d = nc.dram_tensor("xt", [BPC, P, KD, L], bf16, kind="ExternalInput")
    w_d = nc.dram_tensor("w", [P, KD, CO], bf16, kind="ExternalInput")
    wt_d = nc.dram_tensor("wt", [P, MT, D], bf16, kind="ExternalInput")
    biasp_d = nc.dram_tensor("biasp", [P, O], f32, kind="ExternalInput")
    maskbig_d = nc.dram_tensor("maskbig", [P, MT * GI * C], bf16, kind="ExternalInput")
    em_d = nc.dram_tensor("em", [P, 8], f32, kind="ExternalInput")
    permy_d = nc.dram_tensor("permy", [P, P], bf16, kind="ExternalInput")
    permv_d = nc.dram_tensor("permv", [P, P], bf16, kind="ExternalInput")
    ident_d = nc.dram_tensor("ident", [P, P], bf16, kind="ExternalInput")
    cunif_d = nc.dram_tensor("cunif", [P, C], bf16, kind="ExternalInput")
    ones1_d = nc.dram_tensor("ones1", [P, 1], bf16, kind="ExternalInput")
    onesl_d = nc.dram_tensor("onesl", [1, 512], bf16, kind="ExternalInput")
    out_d = nc.dram_tensor("v", [BPC * C, O], f32, kind="ExternalOutput")

    with tile.TileContext(nc) as tc, contextlib.ExitStack() as glb:
        const_pool = glb.enter_context(tc.tile_pool(name="consts", bufs=1))
        x_pool = glb.enter_context(tc.tile_pool(name="x", bufs=BPC))
        xt_pool = glb.enter_context(tc.tile_pool(name="xt", bufs=GI))
        w_pool = glb.enter_context(tc.tile_pool(name="w", bufs=1))
        wt_pool = glb.enter_context(tc.tile_pool(name="wt", bufs=1))
        sm_pool = glb.enter_context(tc.tile_pool(name="sm", bufs=2))
        st_pool = glb.enter_context(tc.tile_pool(name="st", bufs=2))
        # PSUM pools (16KB/partition budget):
        py = glb.enter_context(tc.tile_pool(name="py", bufs=2, space="PSUM"))
        ps = glb.enter_context(tc.tile_pool(name="ps", bufs=1, space="PSUM"))
        pwv = glb.enter_context(tc.tile_pool(name="pwv", bufs=1, space="PSUM"))
        pd = glb.enter_context(tc.tile_pool(name="pd", bufs=2, space="PSUM"))
        ptr = glb.enter_context(tc.tile_pool(name="ptr", bufs=1, space="PSUM"))
        ptiny = glb.enter_context(tc.tile_pool(name="ptiny", bufs=1, space="PSUM"))

        # ---- constants ----
        ident = const_pool.tile([P, P], bf16, name="ident")
        nc.sync.dma_start(ident[:], ident_d[:])
        permy = const_pool.tile([P, P], bf16, name="permy")
        nc.sync.dma_start(permy[:], permy_d[:])
        permv = const_pool.tile([P, P], bf16, name="permv")
        nc.sync.dma_start(permv[:], permv_d[:])
        cunif = const_pool.tile([P, C], bf16, name="cunif")
        nc.sync.dma_start(cunif[:], cunif_d[:])
        biasp = const_pool.tile([P, O], f32, name="biasp")
        nc.sync.dma_start(biasp[:], biasp_d[:])
        maskbig = const_pool.tile([P, MT * GI * C], bf16, name="maskbig")
        nc.sync.dma_start(maskbig[:], maskbig_d[:])
        em = const_pool.tile([P, 8], f32, name="em")
        nc.sync.dma_start(em[:], em_d[:])
        ones1 = const_pool.tile([P, 1], bf16, name="ones1")
        nc.sync.dma_start(ones1[:], ones1_d[:])
        onesl = const_pool.tile([1, 512], bf16, name="onesl")
        nc.sync.dma_start(onesl[:], onesl_d[:])
        eps_sb = const_pool.tile([P, 1], f32, name="eps_sb")
        nc.vector.memset(eps_sb[:], 1e-8)

        # DMA issue order tracks first-use order: g0 x (y-pass) interleaved
        # with w (s-cross), wt (Wv), g1 x. g0's xt is derived on-chip via
        # XBAR DMA-transpose (no HBM traffic); g1 xt reloads into g0's
        # slots mid-run.
        x_sb = []
        for i in range(BPC):
            t = x_pool.tile([P, LT, D], bf16, tag="x", name=f"x{i}")
            x_sb.append(t)
        w_sb = w_pool.tile([P, KD, CO], bf16, name="w_sb")
        wt_sb = wt_pool.tile([P, MT, D], bf16, name="wt_sb")
        for i in range(GI):
            nc.sync.dma_start(x_sb[i][:, 0, :], x_d[i, :, 0, :])
        for k in range(4):
            nc.sync.dma_start(w_sb[:, k, :], w_d[:, k, :])
        for lt in range(1, LT):
            for i in range(GI):
                nc.sync.dma_start(x_sb[i][:, lt, :], x_d[i, :, lt, :])
        for k in range(4, KD):
            nc.sync.dma_start(w_sb[:, k, :], w_d[:, k, :])
        for t in range(0, MT, 4):
            nc.sync.dma_start(wt_sb[:, t:t + 4, :], wt_d[:, t:t + 4, :])
        xt_sb = {}

        def load_xt(i, q=nc.sync):
            t = xt_pool.tile([P, KD, L], bf16, tag="xt", name=f"xt{i}")
            q.dma_start(t[:], xt_d[i])
            xt_sb[i] = t

        for i in range(GI):
            load_xt(i)
        for i in range(GI, BPC):
            nc.sync.dma_start(x_sb[i][:], x_d[i])

        # ---- PE warmup (keep HAM busy while big DMAs land) ----
        wu = ps.tile([P, 512], f32, tag="s", name="warmup")
        for r in range(8):
            nc.tensor.matmul(wu[:], ident[:], maskbig[:, 0:512],
                             start=True, stop=True)

        # ---- per-group state ----
        b_tile = {}
        for g in range(NGRP):
            b_tile[g] = st_pool.tile([P, LT, P], f32, tag=f"b{g}",
                                     bufs=1, name=f"b_g{g}")
            nc.vector.memset(b_tile[g][:], 0.0)

        c_cur = {}
        c_ci = {}

        def make_phases(g, it):
            ii = [g * GI + k for k in range(GI)]  # global b.e. ids
            st = {}

            def tap(name, ap):
                if DBG and g == 0 and it == DBG_IT:
                    nc.sync.dma_start(dbg_d[name][:], ap)

            def phA():
                # ---- y-pass (+ colsum) + yT ----
                ct = c_cur.get(g)
                y_sb = sm_pool.tile([P, D], bf16, tag="y", name=f"y_g{g}_{it}")
                for bank in range(DB):
                    yps = py.tile([P, 512], f32, tag="y",
                                  name=f"yps_g{g}_{it}_{bank}")
                    for lt in range(LT):
                        for i in range(GI):
                            lhsT = cunif[:] if it == 0 else ct[:, lt, i, :]
                            nc.tensor.matmul(
                                yps[i * C:(i + 1) * C, :],
                                lhsT,
                                x_sb[ii[i]][:, lt, bank * 512:(bank + 1) * 512],
                                start=(lt == 0), stop=(lt == LT - 1),
                                tile_position=(0, i * C),
                            )
                    nc.scalar.copy(y_sb[:, bank * 512:(bank + 1) * 512], yps[:])
                tap("dbg_y0", y_sb[:])
                if it > 0:
                    cci = c_ci[g]
                    csps = ptiny.tile([P, 1], f32, tag="tn", name=f"cs_g{g}_{it}")
                    for lt in range(LT):
                        nc.tensor.matmul(
                            csps[:], cci[:, lt, :, :], ones1[:],
                            start=(lt == 0), stop=(lt == LT - 1),
                        )
                    colsum = sm_pool.tile([P, 1], f32, tag="colsum",
                                          name=f"colsum_g{g}_{it}")
                    nc.vector.tensor_copy(colsum[:], csps[:])
                    tap("dbg_cs", colsum[:])
                    st["colsum"] = colsum
                ytps = ptr.tile([P, D], bf16, tag="tr", name=f"ytps_g{g}_{it}")
                for dc in range(KD):
                    nc.tensor.matmul(
                        ytps[:, dc * P:(dc + 1) * P],
                        y_sb[:, dc * P:(dc + 1) * P],
                        permy[:],
                        is_transpose=True,
                        start=(dc == 0), stop=(dc == KD - 1),
                    )
                yt_sb = sm_pool.tile([P, KD, P], bf16, tag="yt",
                                     name=f"yt_g{g}_{it}")
                nc.vector.tensor_copy(
                    yt_sb[:], ytps[:].rearrange("p (k q) -> p k q", k=KD))
                tap("dbg_yt0", yt_sb[:])
                st["yt_sb"] = yt_sb

            def phB():
                # ---- s-cross + extract + squash (+ output on last iter) ----
                yt_sb = st["yt_sb"]
                sps = ps.tile([P, 512], f32, tag="s", name=f"sps_g{g}_{it}")
                for dc in range(KD):
                    for n in range(NB):
                        nc.tensor.matmul(
                            sps[n * C:(n + 1) * C, :],
                            yt_sb[:, dc, n * C:(n + 1) * C],
                            w_sb[:, dc, n * 512:(n + 1) * 512],
                            start=(dc == 0), stop=(dc == KD - 1),
                            tile_position=(0, n * C),
                        )
                tmpb = sm_pool.tile([P, 8, O], bf16, tag="tmpb", bufs=1,
                                    name=f"tmpb_g{g}_{it}")
                nc.vector.tensor_tensor(
                    tmpb[:],
                    sps[:].rearrange("p (cb o) -> p cb o", cb=8),
                    em[:].unsqueeze(2).broadcast_to((P, 8, O)),
                    ALU.mult,
                )
                s0 = sm_pool.tile([P, O], f32, tag="s0", name=f"s0_g{g}_{it}")
                nc.vector.tensor_reduce(
                    s0[:], tmpb[:].rearrange("p cb o -> p o cb"), AX.X, ALU.add)
                s_sb = sm_pool.tile([P, O], f32, tag="ssb", name=f"ssb_g{g}_{it}")
                nc.vector.scalar_tensor_tensor(
                    s_sb[:], biasp[:],
                    16.0 if it == 0 else st["colsum"][:, 0:1],
                    s0[:], ALU.mult, ALU.add,
                )
                tap("dbg_s0", s_sb[:])
                ssq = sm_pool.tile([P, O], f32, tag="ssq", name=f"ssq_g{g}_{it}")
                sq = sm_pool.tile([P, 1], f32, tag="sq", name=f"sq_g{g}_{it}")
                nc.vector.scalar_tensor_tensor(
                    ssq[:], s_sb[:], 1.0, s_sb[:], ALU.mult, ALU.mult,
                    accum_out=sq[:])
                r1 = sm_pool.tile([P, 1], f32, tag="r1", name=f"r1_g{g}_{it}")
                nc.scalar.activation(r1[:], sq[:], AF.Sqrt, bias=eps_sb[:])
                r2 = sm_pool.tile([P, 1], f32, tag="r2", name=f"r2_g{g}_{it}")
                nc.vector.scalar_tensor_tensor(
                    r2[:], sq[:], 1.0, r1[:], ALU.add, ALU.mult)
                rr = sm_pool.tile([P, 1], f32, tag="rr", name=f"rr_g{g}_{it}")
                nc.vector.reciprocal(rr[:], r2[:])
                vdt = f32 if it == ITERS - 1 else bf16
                v_sb = sm_pool.tile([P, O], vdt, tag="v", name=f"v_g{g}_{it}")
                nc.vector.tensor_scalar(
                    v_sb[:], s_sb[:], sq[:], rr[:], ALU.mult, ALU.mult)
                tap("dbg_v0", v_sb[:])
                st["v_sb"] = v_sb
                if it == ITERS - 1:
                    # raw partition order (rows 4c+i); host reorders to (i, c)
                    nc.scalar.dma_start(out_d[g * P:(g + 1) * P, :], v_sb[:])

            def phC1():
                # ---- vT + vflat + vsel (mostly non-PE chain) ----
                if it == ITERS - 1:
                    return
                vb = st["v_sb"]
                vtps = ptiny.tile([O, P], bf16, tag="tn", name=f"vtps_g{g}_{it}")
                nc.tensor.matmul(vtps[:], vb[:], permv[:], is_transpose=True,
                                 start=True, stop=True)
                vt_sb = sm_pool.tile([O, P], bf16, tag="vts",
                                     name=f"vts_g{g}_{it}")
                nc.vector.tensor_copy(vt_sb[:], vtps[:])
                vflat4 = sm_pool.tile([P, GI, MT], bf16, tag="vf4",
                                      name=f"vf4_g{g}_{it}")
                nc.vector.tensor_copy(
                    vflat4[0:O, :, :],
                    vt_sb[:, 0:O].rearrange("p (i t) -> p i t", i=GI))
                nc.gpsimd.dma_start(vflat4[O:P, :, :], vt_sb[:, O:P])
                vsel_all = sm_pool.tile([P, MT, GI, C], bf16, tag="vsel",
                                        bufs=1, name=f"vsel_g{g}_{it}")
                mbv = maskbig[:].rearrange("p (t i c) -> p t i c", t=MT, i=GI)
                vfv = vflat4[:].rearrange("p i t -> p t i")
                for q in range(4):
                    ts = slice(4 * q, 4 * q + 4)
                    nc.vector.tensor_tensor(
                        vsel_all[:, ts, :, :],
                        mbv[:, ts, :, :],
                        vfv[:, ts].unsqueeze(3).broadcast_to((P, 4, GI, C)),
                        ALU.mult,
                    )
                if DBG and g == 0 and it == DBG_IT:
                    nc.sync.dma_start(dbg_d["dbg_vf0"][:],
                                      vflat4[:].rearrange("p i t -> p t i")[:, :, 0])
                    nc.sync.dma_start(dbg_d["dbg_vs0"][:], vsel_all[:, :, 0, :])
                st["vsel_all"] = vsel_all

            def phC2():
                # ---- Wv + WvT + bcorr ----
                if it == ITERS - 1:
                    return
                v_sb = st["v_sb"]
                vsel_all = st["vsel_all"]
                wv_sb = sm_pool.tile([P, D], bf16, tag="wv", name=f"wv_g{g}_{it}")
                for bank in range(DB):
                    wvps = pwv.tile([P, 512], f32, tag="wv",
                                    name=f"wvps_g{g}_{it}_{bank}")
                    for t in range(MT):
                        nc.tensor.matmul(
                            wvps[:],
                            vsel_all[:, t, :, :],
                            wt_sb[:, t, bank * 512:(bank + 1) * 512],
                            start=(t == 0), stop=(t == MT - 1),
                        )
                    nc.scalar.copy(wv_sb[:, bank * 512:(bank + 1) * 512], wvps[:])
                wvtps = ptr.tile([P, D], bf16, tag="tr", name=f"wvtps_g{g}_{it}")
                for dc in range(KD):
                    nc.tensor.matmul(
                        wvtps[:, dc * P:(dc + 1) * P],
                        wv_sb[:, dc * P:(dc + 1) * P],
                        ident[:],
                        is_transpose=True,
                        start=(dc == 0), stop=(dc == KD - 1),
                    )
                wvt_sb = sm_pool.tile([P, KD, P], bf16, tag="wvt",
                                      name=f"wvt_g{g}_{it}")
                nc.scalar.copy(wvt_sb[:],
                               wvtps[:].rearrange("p (k q) -> p k q", k=KD))
                tap("dbg_wv0", wv_sb[:])
                tap("dbg_wvt0", wvt_sb[:])
                st["wvt_sb"] = wvt_sb
                bcsc = sm_pool.tile([P, O], f32, tag="bcsc",
                                    name=f"bcsc_g{g}_{it}")
                bcp = sm_pool.tile([P, 1], bf16, tag="bcp", name=f"bcp_g{g}_{it}")
                nc.vector.scalar_tensor_tensor(
                    bcsc[:], v_sb[:], 1.0, biasp[:], ALU.mult, ALU.mult,
                    accum_out=bcp[:])
                bcrps = ptiny.tile([1, P], bf16, tag="tn",
                                   name=f"bcrps_g{g}_{it}")
                nc.tensor.matmul(bcrps[:], bcp[:], ident[:], is_transpose=True,
                                 start=True, stop=True)
                bcrow = sm_pool.tile([1, P], bf16, tag="bcrow",
                                     name=f"bcrow_g{g}_{it}")
                nc.vector.tensor_copy(bcrow[:], bcrps[:])
                st["bcrow"] = bcrow

            def phD():
                # ---- delta + b update + softmax ----
                if it == ITERS - 1:
                    return
                wvt_sb, bcrow = st["wvt_sb"], st["bcrow"]
                dps = pd.tile([P, 512], f32, tag="d", name=f"dps_g{g}_{it}")
                for dc in range(KD):
                    for i in range(GI):
                        nc.tensor.matmul(
                            dps[i * C:(i + 1) * C, :],
                            wvt_sb[:, dc, i * C:(i + 1) * C],
                            xt_sb[ii[i]][:, dc, :],
                            start=(dc == 0), stop=False,
                            tile_position=(0, i * C),
                        )
                for i in range(GI):
                    nc.tensor.matmul(
                        dps[i * C:(i + 1) * C, :],
                        bcrow[:, i::GI],
                        onesl[:],
                        start=False, stop=True,
                        tile_position=(0, i * C),
                    )
                ds_sb = sm_pool.tile([P, 512], bf16, tag="ds",
                                     name=f"ds_g{g}_{it}")
                nc.scalar.copy(ds_sb[:], dps[:])
                tap("dbg_ds0", ds_sb[:])
                baps = pd.tile([P, 512], bf16, tag="d", name=f"baps_g{g}_{it}")
                for lt in range(LT):
                    nc.tensor.matmul(
                        baps[:, lt * P:(lt + 1) * P],
                        ds_sb[:, lt * P:(lt + 1) * P],
                        ident[:],
                        is_transpose=True,
                        start=(lt == 0), stop=(lt == LT - 1),
                    )
                nc.vector.tensor_tensor(
                    b_tile[g][:], b_tile[g][:],
                    baps[:].rearrange("p (lt q) -> p lt q", lt=LT),
                    ALU.add,
                )
                tap("dbg_b0", b_tile[g][:])
                cexp = sm_pool.tile([P, LT, P], bf16, tag="cexp", bufs=1,
                                    name=f"cexp_g{g}_{it}")
                csum = sm_pool.tile([P, LT, GI], f32, tag="csum",
                                    name=f"csum_g{g}_{it}")
                crec = sm_pool.tile([P, LT, GI], f32, tag="crec",
                                    name=f"crec_g{g}_{it}")
                cnx = sm_pool.tile([P, LT, GI, C], bf16, tag="cnx", bufs=4,
                                   name=f"cnx_g{g}_{it}")
                for h in range(2):
                    lts = slice(2 * h, 2 * h + 2)
                    nc.scalar.activation(cexp[:, lts, :], b_tile[g][:, lts, :],
                                         AF.Exp)
                    nc.vector.tensor_reduce(
                        csum[:, lts, :],
                        cexp[:, lts, :].rearrange("p lt (i c) -> p lt i c",
                                                  i=GI),
                        AX.X, ALU.add)
                    nc.vector.reciprocal(crec[:, lts, :], csum[:, lts, :])
                    nc.vector.tensor_tensor(
                        cnx[:, lts, :, :],
                        cexp[:, lts, :].rearrange("p lt (i c) -> p lt i c",
                                                  i=GI),
                        crec[:, lts, :].unsqueeze(3)
                            .broadcast_to((P, 2, GI, C)),
                        ALU.mult,
                    )
                tap("dbg_c1", cnx[:])
                c_cur[g] = cnx
                # (c, i)-ordered copy for the colsum matmul (off the
                # critical chain -- only needed by next iter's colsum)
                cci = sm_pool.tile([P, LT, C, GI], bf16, tag="cci", bufs=2,
                                   name=f"cci_g{g}_{it}")
                nc.vector.tensor_tensor(
                    cci[:],
                    cexp[:].rearrange("p lt (i c) -> p lt c i", i=GI),
                    crec[:].unsqueeze(2).broadcast_to((P, LT, C, GI)),
                    ALU.mult,
                )
                c_ci[g] = cci

            return [phA, phB, phC1, phC2, phD]

        # software-pipelined schedule: group 1 runs one iteration behind
        # group 0 so each group's serial chain is hidden by the other's
        # matmul phases.
        for f in make_phases(0, 0):
            f()
        for (ita, itb) in [(1, 0), (2, 1)]:
            pa = make_phases(0, ita)
            pb = make_phases(1, itb)
            pa[0](); pa[1](); pb[0](); pa[2](); pb[1]()
            pa[3](); pa[4]()
            if itb == 0:
                for i in range(GI, BPC):
                    load_xt(i)
            pb[2](); pb[3](); pb[4]()
        for f in make_phases(1, 2):
            f()

    nc.compile()
    return nc


_NC_CACHE = None


def _get_nc():
    global _NC_CACHE
    if _NC_CACHE is None:
        _NC_CACHE = build_kernel()
    return _NC_CACHE


def _make_consts():
    ident = np.eye(P, dtype=_BF16)
    # yT perm: out col j=4c+i takes y row 32i+c -> permy[32i+c, 4c+i] = 1
    permy = np.zeros((P, P), dtype=_BF16)
    for c in range(C):
        for i in range(GI):
            permy[32 * i + c, 4 * c + i] = 1
    # vT perm: out col 64j+16i+t takes v row 4(2t+j)+i (contiguous vflat runs)
    permv = np.zeros((P, P), dtype=_BF16)
    for t in range(MT):
        for j in range(2):
            for i in range(GI):
                permv[4 * (2 * t + j) + i, 64 * j + 16 * i + t] = 1
    cunif = np.full((P, C), 1.0 / C, dtype=_BF16)
    # biasp[4c+i, o] = fc_b-independent: filled in kernel() (needs fc_b)
    # maskall[p, t, c'] = (c' == 2t + p//64)
    pp = np.arange(P)
    tt = np.arange(MT)
    cc = np.arange(C)
    maskall = (cc[None, None, :] == (2 * tt[None, :, None] + pp[:, None, None] // O)
               ).astype(np.float32).reshape(P, MT * C).astype(_BF16)
    em = (np.arange(8)[None, :] == (pp[:, None] // 4) % 8).astype(np.float32)
    maskbig = np.broadcast_to(
        maskall.reshape(P, MT, 1, C), (P, MT, GI, C)
    ).reshape(P, MT * GI * C).astype(_BF16)
    ones1 = np.ones((P, 1), dtype=_BF16)
    onesl = np.ones((1, 512), dtype=_BF16)
    return ident, permy, permv, cunif, maskbig, em, ones1, onesl


def kernel(inputs, fc_w, fc_b, _trace=False):
    from concourse.bass_utils import run_bass_kernel_spmd

    if _trace:
        _install_ntff_shim()

    nc = _get_nc()

    ident, permy, permv, cunif, maskbig, em, ones1, onesl = _make_consts()

    xf = np.asarray(inputs, dtype=np.float32)
    # x layout [B, 128, LT, D]: x_l[b, lp, lt, d] = x[b, lt*128+lp, d]
    x_l = np.ascontiguousarray(
        xf.reshape(B, LT, P, D).transpose(0, 2, 1, 3)).astype(_BF16)
    # xt layout [B, 128, KD, L]: xt[b, dp, kd, l] = x[b, l, kd*128+dp]
    xt_l = np.ascontiguousarray(
        xf.transpose(0, 2, 1).reshape(B, KD, P, L).transpose(0, 2, 1, 3)
    ).astype(_BF16)
    wf = np.asarray(fc_w, dtype=np.float32)
    # w layout [128, KD, CO]: w[dp, kd, co] = fc_w[kd*128+dp, co]
    w_l = np.ascontiguousarray(
        wf.reshape(KD, P, CO).transpose(1, 0, 2)).astype(_BF16)
    # wt layout [128, MT, D]: wt[p, t, d] = fc_w[d, t*128+p]
    wt_l = np.ascontiguousarray(
        wf.T.reshape(MT, P, D).transpose(1, 0, 2)).astype(_BF16)
    bf = np.asarray(fc_b, dtype=np.float32).reshape(C, O)
    biasp = np.ascontiguousarray(bf[np.arange(P) // 4, :])

    in_maps = []
    for core in range(NCORES):
        in_maps.append({
            "x": x_l[core * BPC:(core + 1) * BPC],
            "xt": xt_l[core * BPC:(core + 1) * BPC],
            "w": w_l,
            "wt": wt_l,
            "biasp": biasp,
            "maskbig": maskbig,
            "em": em,
            "permy": permy,
            "permv": permv,
            "ident": ident,
            "cunif": cunif,
            "ones1": ones1,
            "onesl": onesl,
        })

    res = run_bass_kernel_spmd(
        nc, in_maps, core_ids=list(range(NCORES)), trace=_trace,
    )
    out = np.concatenate(
        [res.results[core]["v"].reshape(NGRP, C, GI, O)
         .transpose(0, 2, 1, 3).reshape(BPC, C, O)
         for core in range(NCORES)],
        axis=0,
    )
    if _trace:
        kernel.last_exec_time_ns = res.exec_time_ns
        kernel.last_results = res
    return out


# revision 55
# speedup vs baseline: 1.2363x; 1.2363x over previous
"""Trainium2 Bass kernel for nn_CapsuleLayer (B=64, L=512, D=1024, C=32, O=64).

u-hat-free formulation. The reference computes u_hat = x @ fc_w (B*L*D*CO
MACs -- 218us/core of PE at bf16) and then routes. But routing only ever
consumes u_hat through two contractions:

    s_j[c,o]   = sum_l c_ij[l,c] * u[l,c,o] = ((c_ij^T @ x) @ fc_w)[c, (c,o)]
    delta[l,c] = sum_o u[l,c,o] * v[c,o]   = (x @ (fc_w . v))[l, c]

so u_hat never needs to be materialized.  Per iteration and batch element:
    y    = c^T x                  (C x D;   PE, 4-way col-tiled)
    s    = diag(y @ w) + colsum(c)*bias     (PE cross + DVE extract)
    v    = squash(s)
    Wv   = sum_o w[d,(c,o)] v[c,o]          (PE via block-diag vsel weights)
    dT   = Wv^T @ x^T + bcorr               (PE, 4-way col-tiled)
    b   += dT^T ; c = softmax(b)
All matmuls keep M=128 via tile_position col-tiling of 4 batch elements.
Data-parallel over batch: 8 cores x 8 batch elements (2 groups of 4).
"""

import contextlib
import ctypes
import sys
import types

import numpy as np
import ml_dtypes

DBG = False
DBG_IT = 0

B, L, D = 64, 512, 1024
C, O = 32, 64
CO = C * O
ITERS = 3
NCORES = 8
BPC = B // NCORES           # 8 batch elements per core
GI = 4                      # batch elements per group (col-tiling width)
NGRP = BPC // GI            # 2
P = 128
LT = L // P                 # 4 l-chunks
KD = D // P                 # 8 d-chunks
DB = D // 512               # 2 d-banks
MT = CO // P                # 16 co-chunks
NB = CO // 512              # 4 co-banks

_BF16 = ml_dtypes.bfloat16

# ---------------------------------------------------------------------------
# NTFF profiling shim (used when tracing is requested by the test harness)
# ---------------------------------------------------------------------------


def _install_ntff_shim():
    if "antenv.axon_hooks" in sys.modules:
        return
    so_path = "/opt/axon/libaxon_pjrt.so"
    hook = None
    try:
        lib = ctypes.CDLL(so_path)
        if hasattr(lib, "axon_start_nrt_profile"):
            lib.axon_start_nrt_profile.argtypes = [
                ctypes.POINTER(ctypes.c_int64),
                ctypes.c_size_t,
            ]
            lib.axon_start_nrt_profile.restype = ctypes.c_int64
            lib.axon_stop_nrt_profile.argtypes = [ctypes.c_char_p]
            lib.axon_stop_nrt_profile.restype = ctypes.c_int64

            @contextlib.contextmanager
            def hook(output_dir, device_ids):
                import jax

                jax.devices()
                if device_ids:
                    ids = (ctypes.c_int64 * len(device_ids))(*device_ids)
                    rc = lib.axon_start_nrt_profile(ids, len(device_ids))
                else:
                    rc = lib.axon_start_nrt_profile(None, 0)
                if rc != 0:
                    raise RuntimeError(f"axon_start_nrt_profile rc={rc}")
                try:
                    yield
                finally:
                    n = lib.axon_stop_nrt_profile(str(output_dir).encode())
                    if n < 0:
                        raise RuntimeError(f"axon_stop_nrt_profile rc={n}")
    except OSError:
        pass
    mod = types.ModuleType("antenv.axon_hooks")
    mod.get_axon_ntff_profile_hook = lambda: hook
    mod.set_axon_ntff_profile_hook = lambda h: None
    sys.modules["antenv.axon_hooks"] = mod

    import concourse.bass_utils as bu

    bu.upload_artifacts = lambda tmpdir: tmpdir


# ---------------------------------------------------------------------------
# Kernel builder
# ---------------------------------------------------------------------------


def build_kernel():
    import concourse.bacc as bacc
    import concourse.tile as tile
    import concourse.mybir as mybir

    f32 = mybir.dt.float32
    bf16 = mybir.dt.bfloat16
    AF = mybir.ActivationFunctionType
    ALU = mybir.AluOpType
    AX = mybir.AxisListType

    nc = bacc.Bacc("TRN2", target_bir_lowering=False, debug=False)

    dbg_d = {}
    if DBG:
        def _dbg(name, shape, dt):
            dbg_d[name] = nc.dram_tensor(name, shape, dt, kind="ExternalOutput")
        _dbg("dbg_y0", [P, D], bf16)
        _dbg("dbg_yt0", [P, KD, P], bf16)
        _dbg("dbg_s0", [P, O], f32)
        _dbg("dbg_v0", [P, O], f32)
        _dbg("dbg_vt0", [O, P], bf16)
        _dbg("dbg_vf0", [P, MT], bf16)
        _dbg("dbg_vs0", [P, MT, C], bf16)
        _dbg("dbg_wv0", [P, D], bf16)
        _dbg("dbg_wvt0", [P, KD, P], bf16)
        _dbg("dbg_ds0", [P, 512], bf16)
        _dbg("dbg_b0", [P, LT, P], f32)
        _dbg("dbg_c1", [P, LT, GI, C], bf16)
        _dbg("dbg_cs", [P, 1], f32)

    x_d = nc.dram_tensor("x", [BPC, P, LT, D], bf16, kind="ExternalInput")
    xt_d = nc.dram_tensor("xt", [BPC, P, KD, L], bf16, kind="ExternalInput")
    w_d = nc.dram_tensor("w", [P, KD, CO], bf16, kind="ExternalInput")
    wt_d = nc.dram_tensor("wt", [P, MT, D], bf16, kind="ExternalInput")
    biasp_d = nc.dram_tensor("biasp", [P, O], f32, kind="ExternalInput")
    maskbig_d = nc.dram_tensor("maskbig", [P, MT * GI * C], bf16, kind="ExternalInput")
    em_d = nc.dram_tensor("em", [P, 8], f32, kind="ExternalInput")
    permy_d = nc.dram_tensor("permy", [P, P], bf16, kind="ExternalInput")
    permv_d = nc.dram_tensor("permv", [P, P], bf16, kind="ExternalInput")
    ident_d = nc.dram_tensor("ident", [P, P], bf16, kind="ExternalInput")
    cunif_d = nc.dram_tensor("cunif", [P, C], bf16, kind="ExternalInput")
    ones1_d = nc.dram_tensor("ones1", [P, 1], bf16, kind="ExternalInput")
    onesl_d = nc.dram_tensor("onesl", [1, 512], bf16, kind="ExternalInput")
    out_d = nc.dram_tensor("v", [BPC * C, O], f32, kind="ExternalOutput")

    with tile.TileContext(nc) as tc, contextlib.ExitStack() as glb:
        const_pool = glb.enter_context(tc.tile_pool(name="consts", bufs=1))
        x_pool = glb.enter_context(tc.tile_pool(name="x", bufs=BPC))
        xt_pool = glb.enter_context(tc.tile_pool(name="xt", bufs=GI))
        w_pool = glb.enter_context(tc.tile_pool(name="w", bufs=1))
        wt_pool = glb.enter_context(tc.tile_pool(name="wt", bufs=1))
        sm_pool = glb.enter_context(tc.tile_pool(name="sm", bufs=2))
        st_pool = glb.enter_context(tc.tile_pool(name="st", bufs=2))
        # PSUM pools (16KB/partition budget):
        py = glb.enter_context(tc.tile_pool(name="py", bufs=2, space="PSUM"))
        ps = glb.enter_context(tc.tile_pool(name="ps", bufs=1, space="PSUM"))
        pwv = glb.enter_context(tc.tile_pool(name="pwv", bufs=1, space="PSUM"))
        pd = glb.enter_context(tc.tile_pool(name="pd", bufs=2, space="PSUM"))
        ptr = glb.enter_context(tc.tile_pool(name="ptr", bufs=1, space="PSUM"))
        ptiny = glb.enter_context(tc.tile_pool(name="ptiny", bufs=1, space="PSUM"))

        # ---- constants ----
        ident = const_pool.tile([P, P], bf16, name="ident")
        nc.sync.dma_start(ident[:], ident_d[:])
        permy = const_pool.tile([P, P], bf16, name="permy")
        nc.sync.dma_start(permy[:], permy_d[:])
        permv = const_pool.tile([P, P], bf16, name="permv")
        nc.sync.dma_start(permv[:], permv_d[:])
        cunif = const_pool.tile([P, C], bf16, name="cunif")
        nc.sync.dma_start(cunif[:], cunif_d[:])
        biasp = const_pool.tile([P, O], f32, name="biasp")
        nc.sync.dma_start(biasp[:], biasp_d[:])
        maskbig = const_pool.tile([P, MT * GI * C], bf16, name="maskbig")
        nc.sync.dma_start(maskbig[:], maskbig_d[:])
        em = const_pool.tile([P, 8], f32, name="em")
        nc.sync.dma_start(em[:], em_d[:])
        ones1 = const_pool.tile([P, 1], bf16, name="ones1")
        nc.sync.dma_start(ones1[:], ones1_d[:])
        onesl = const_pool.tile([1, 512], bf16, name="onesl")
        nc.sync.dma_start(onesl[:], onesl_d[:])
        eps_sb = const_pool.tile([P, 1], f32, name="eps_sb")
        nc.vector.memset(eps_sb[:], 1e-8)

        # DMA issue order tracks first-use order: g0 x (y-pass) interleaved
        # with w (s-cross), wt (Wv), g1 x. g0's xt is derived on-chip via
        # XBAR DMA-transpose (no HBM traffic); g1 xt reloads into g0's
        # slots mid-run.
        x_sb = []
        for i in range(BPC):
            t = x_pool.tile([P, LT, D], bf16, tag="x", name=f"x{i}")
            x_sb.append(t)
        w_sb = w_pool.tile([P, KD, CO], bf16, name="w_sb")
        wt_sb = wt_pool.tile([P, MT, D], bf16, name="wt_sb")
        for i in range(GI):
            nc.sync.dma_start(x_sb[i][:, 0, :], x_d[i, :, 0, :])
        for k in range(4):
            nc.sync.dma_start(w_sb[:, k, :], w_d[:, k, :])
        for lt in range(1, LT):
            for i in range(GI):
                nc.sync.dma_start(x_sb[i][:, lt, :], x_d[i, :, lt, :])
        for k in range(4, KD):
            nc.sync.dma_start(w_sb[:, k, :], w_d[:, k, :])
        for t in range(0, MT, 4):
            nc.sync.dma_start(wt_sb[:, t:t + 4, :], wt_d[:, t:t + 4, :])
        xt_sb = {}

        def load_xt(i, q=nc.sync):
            t = xt_pool.tile([P, KD, L], bf16, tag="xt", name=f"xt{i}")
            q.dma_start(t[:], xt_d[i])
            xt_sb[i] = t

        for i in range(GI):
            load_xt(i)
        for i in range(GI, BPC):
            nc.sync.dma_start(x_sb[i][:], x_d[i])

        # ---- PE warmup (keep HAM busy while big DMAs land) ----
        wu = ps.tile([P, 512], f32, tag="s", name="warmup")
        for r in range(16):
            nc.tensor.matmul(wu[:], ident[:], maskbig[:, 0:512],
                             start=True, stop=True)

        # ---- per-group state ----
        b_tile = {}
        for g in range(NGRP):
            b_tile[g] = st_pool.tile([P, LT, P], f32, tag=f"b{g}",
                                     bufs=1, name=f"b_g{g}")
            nc.vector.memset(b_tile[g][:], 0.0)

        c_cur = {}
        c_ci = {}

        def make_phases(g, it):
            ii = [g * GI + k for k in range(GI)]  # global b.e. ids
            st = {}

            def tap(name, ap):
                if DBG and g == 0 and it == DBG_IT:
                    nc.sync.dma_start(dbg_d[name][:], ap)

            def phA():
                # ---- y-pass (+ colsum) + yT ----
                ct = c_cur.get(g)
                y_sb = sm_pool.tile([P, D], bf16, tag="y", name=f"y_g{g}_{it}")
                for bank in range(DB):
                    yps = py.tile([P, 512], f32, tag="y",
                                  name=f"yps_g{g}_{it}_{bank}")
                    for lt in range(LT):
                        for i in range(GI):
                            lhsT = cunif[:] if it == 0 else ct[:, lt, i, :]
                            nc.tensor.matmul(
                                yps[i * C:(i + 1) * C, :],
                                lhsT,
                                x_sb[ii[i]][:, lt, bank * 512:(bank + 1) * 512],
                                start=(lt == 0), stop=(lt == LT - 1),
                                tile_position=(0, i * C),
                            )
                    nc.scalar.copy(y_sb[:, bank * 512:(bank + 1) * 512], yps[:])
                tap("dbg_y0", y_sb[:])
                if it > 0:
                    cci = c_ci[g]
                    csps = ptiny.tile([P, 1], f32, tag="tn", name=f"cs_g{g}_{it}")
                    for lt in range(LT):
                        nc.tensor.matmul(
                            csps[:], cci[:, lt, :, :], ones1[:],
                            start=(lt == 0), stop=(lt == LT - 1),
                        )
                    colsum = sm_pool.tile([P, 1], f32, tag="colsum",
                                          name=f"colsum_g{g}_{it}")
                    nc.vector.tensor_copy(colsum[:], csps[:])
                    tap("dbg_cs", colsum[:])
                    st["colsum"] = colsum
                ytps = ptr.tile([P, D], bf16, tag="tr", name=f"ytps_g{g}_{it}")
                for dc in range(KD):
                    nc.tensor.matmul(
                        ytps[:, dc * P:(dc + 1) * P],
                        y_sb[:, dc * P:(dc + 1) * P],
                        permy[:],
                        is_transpose=True,
                        start=(dc == 0), stop=(dc == KD - 1),
                    )
                yt_sb = sm_pool.tile([P, KD, P], bf16, tag="yt",
                                     name=f"yt_g{g}_{it}")
                nc.vector.tensor_copy(
                    yt_sb[:], ytps[:].rearrange("p (k q) -> p k q", k=KD))
                tap("dbg_yt0", yt_sb[:])
                st["yt_sb"] = yt_sb

            def phB():
                # ---- s-cross + extract + squash (+ output on last iter) ----
                yt_sb = st["yt_sb"]
                sps = ps.tile([P, 512], f32, tag="s", name=f"sps_g{g}_{it}")
                for dc in range(KD):
                    for n in range(NB):
                        nc.tensor.matmul(
                            sps[n * C:(n + 1) * C, :],
                            yt_sb[:, dc, n * C:(n + 1) * C],
                            w_sb[:, dc, n * 512:(n + 1) * 512],
                            start=(dc == 0), stop=(dc == KD - 1),
                            tile_position=(0, n * C),
                        )
                tmpb = sm_pool.tile([P, 8, O], bf16, tag="tmpb", bufs=1,
                                    name=f"tmpb_g{g}_{it}")
                nc.vector.tensor_tensor(
                    tmpb[:],
                    sps[:].rearrange("p (cb o) -> p cb o", cb=8),
                    em[:].unsqueeze(2).broadcast_to((P, 8, O)),
                    ALU.mult,
                )
                s0 = sm_pool.tile([P, O], f32, tag="s0", name=f"s0_g{g}_{it}")
                nc.vector.tensor_reduce(
                    s0[:], tmpb[:].rearrange("p cb o -> p o cb"), AX.X, ALU.add)
                s_sb = sm_pool.tile([P, O], f32, tag="ssb", name=f"ssb_g{g}_{it}")
                nc.vector.scalar_tensor_tensor(
                    s_sb[:], biasp[:],
                    16.0 if it == 0 else st["colsum"][:, 0:1],
                    s0[:], ALU.mult, ALU.add,
                )
                tap("dbg_s0", s_sb[:])
                ssq = sm_pool.tile([P, O], f32, tag="ssq", name=f"ssq_g{g}_{it}")
                sq = sm_pool.tile([P, 1], f32, tag="sq", name=f"sq_g{g}_{it}")
                nc.vector.scalar_tensor_tensor(
                    ssq[:], s_sb[:], 1.0, s_sb[:], ALU.mult, ALU.mult,
                    accum_out=sq[:])
                r1 = sm_pool.tile([P, 1], f32, tag="r1", name=f"r1_g{g}_{it}")
                nc.scalar.activation(r1[:], sq[:], AF.Sqrt, bias=eps_sb[:])
                r2 = sm_pool.tile([P, 1], f32, tag="r2", name=f"r2_g{g}_{it}")
                nc.vector.scalar_tensor_tensor(
                    r2[:], sq[:], 1.0, r1[:], ALU.add, ALU.mult)
                rr = sm_pool.tile([P, 1], f32, tag="rr", name=f"rr_g{g}_{it}")
                nc.vector.reciprocal(rr[:], r2[:])
                vdt = f32 if it == ITERS - 1 else bf16
                v_sb = sm_pool.tile([P, O], vdt, tag="v", name=f"v_g{g}_{it}")
                nc.vector.tensor_scalar(
                    v_sb[:], s_sb[:], sq[:], rr[:], ALU.mult, ALU.mult)
                tap("dbg_v0", v_sb[:])
                st["v_sb"] = v_sb
                if it == ITERS - 1:
                    # raw partition order (rows 4c+i); host reorders to (i, c)
                    nc.scalar.dma_start(out_d[g * P:(g + 1) * P, :], v_sb[:])

            def phC1():
                # ---- vT + vflat + vsel (mostly non-PE chain) ----
                if it == ITERS - 1:
                    return
                vb = st["v_sb"]
                vtps = ptiny.tile([O, P], bf16, tag="tn", name=f"vtps_g{g}_{it}")
                nc.tensor.matmul(vtps[:], vb[:], permv[:], is_transpose=True,
                                 start=True, stop=True)
                vt_sb = sm_pool.tile([O, P], bf16, tag="vts",
                                     name=f"vts_g{g}_{it}")
                nc.vector.tensor_copy(vt_sb[:], vtps[:])
                vflat4 = sm_pool.tile([P, GI, MT], bf16, tag="vf4",
                                      name=f"vf4_g{g}_{it}")
                nc.vector.tensor_copy(
                    vflat4[0:O, :, :],
                    vt_sb[:, 0:O].rearrange("p (i t) -> p i t", i=GI))
                nc.gpsimd.dma_start(vflat4[O:P, :, :], vt_sb[:, O:P])
                vsel_all = sm_pool.tile([P, MT, GI, C], bf16, tag="vsel",
                                        bufs=1, name=f"vsel_g{g}_{it}")
                mbv = maskbig[:].rearrange("p (t i c) -> p t i c", t=MT, i=GI)
                vfv = vflat4[:].rearrange("p i t -> p t i")
                for q in range(4):
                    ts = slice(4 * q, 4 * q + 4)
                    nc.vector.tensor_tensor(
                        vsel_all[:, ts, :, :],
                        mbv[:, ts, :, :],
                        vfv[:, ts].unsqueeze(3).broadcast_to((P, 4, GI, C)),
                        ALU.mult,
                    )
                if DBG and g == 0 and it == DBG_IT:
                    nc.sync.dma_start(dbg_d["dbg_vf0"][:],
                                      vflat4[:].rearrange("p i t -> p t i")[:, :, 0])
                    nc.sync.dma_start(dbg_d["dbg_vs0"][:], vsel_all[:, :, 0, :])
                st["vsel_all"] = vsel_all

            def phC2():
                # ---- Wv + WvT + bcorr ----
                if it == ITERS - 1:
                    return
                v_sb = st["v_sb"]
                vsel_all = st["vsel_all"]
                wv_sb = sm_pool.tile([P, D], bf16, tag="wv", name=f"wv_g{g}_{it}")
                for bank in range(DB):
                    wvps = pwv.tile([P, 512], f32, tag="wv",
                                    name=f"wvps_g{g}_{it}_{bank}")
                    for t in range(MT):
                        nc.tensor.matmul(
                            wvps[:],
                            vsel_all[:, t, :, :],
                            wt_sb[:, t, bank * 512:(bank + 1) * 512],
                            start=(t == 0), stop=(t == MT - 1),
                        )
                    nc.scalar.copy(wv_sb[:, bank * 512:(bank + 1) * 512], wvps[:])
                wvtps = ptr.tile([P, D], bf16, tag="tr", name=f"wvtps_g{g}_{it}")
                for dc in range(KD):
                    nc.tensor.matmul(
                        wvtps[:, dc * P:(dc + 1) * P],
                        wv_sb[:, dc * P:(dc + 1) * P],
                        ident[:],
                        is_transpose=True,
                        start=(dc == 0), stop=(dc == KD - 1),
                    )
                wvt_sb = sm_pool.tile([P, KD, P], bf16, tag="wvt",
                                      name=f"wvt_g{g}_{it}")
                nc.scalar.copy(wvt_sb[:],
                               wvtps[:].rearrange("p (k q) -> p k q", k=KD))
                tap("dbg_wv0", wv_sb[:])
                tap("dbg_wvt0", wvt_sb[:])
                st["wvt_sb"] = wvt_sb
                bcsc = sm_pool.tile([P, O], f32, tag="bcsc",
                                    name=f"bcsc_g{g}_{it}")
                bcp = sm_pool.tile([P, 1], bf16, tag="bcp", name=f"bcp_g{g}_{it}")
                nc.vector.scalar_tensor_tensor(
                    bcsc[:], v_sb[:], 1.0, biasp[:], ALU.mult, ALU.mult,
                    accum_out=bcp[:])
                bcrps = ptiny.tile([1, P], bf16, tag="tn",
                                   name=f"bcrps_g{g}_{it}")
                nc.tensor.matmul(bcrps[:], bcp[:], ident[:], is_transpose=True,
                                 start=True, stop=True)
                bcrow = sm_pool.tile([1, P], bf16, tag="bcrow",
                                     name=f"bcrow_g{g}_{it}")
                nc.vector.tensor_copy(bcrow[:], bcrps[:])
                st["bcrow"] = bcrow

            def phD():
                # ---- delta + b update + softmax ----
                if it == ITERS - 1:
                    return
                wvt_sb, bcrow = st["wvt_sb"], st["bcrow"]
                dps = pd.tile([P, 512], f32, tag="d", name=f"dps_g{g}_{it}")
                for dc in range(KD):
                    for i in range(GI):
                        nc.tensor.matmul(
                            dps[i * C:(i + 1) * C, :],
                            wvt_sb[:, dc, i * C:(i + 1) * C],
                            xt_sb[ii[i]][:, dc, :],
                            start=(dc == 0), stop=False,
                            tile_position=(0, i * C),
                        )
                for i in range(GI):
                    nc.tensor.matmul(
                        dps[i * C:(i + 1) * C, :],
                        bcrow[:, i::GI],
                        onesl[:],
                        start=False, stop=True,
                        tile_position=(0, i * C),
                    )
                ds_sb = sm_pool.tile([P, 512], bf16, tag="ds",
                                     name=f"ds_g{g}_{it}")
                nc.scalar.copy(ds_sb[:], dps[:])
                tap("dbg_ds0", ds_sb[:])
                baps = pd.tile([P, 512], bf16, tag="d", name=f"baps_g{g}_{it}")
                for lt in range(LT):
                    nc.tensor.matmul(
                        baps[:, lt * P:(lt + 1) * P],
                        ds_sb[:, lt * P:(lt + 1) * P],
                        ident[:],
                        is_transpose=True,
                        start=(lt == 0), stop=(lt == LT - 1),
                    )
                nc.vector.tensor_tensor(
                    b_tile[g][:], b_tile[g][:],
                    baps[:].rearrange("p (lt q) -> p lt q", lt=LT),
                    ALU.add,
                )
                tap("dbg_b0", b_tile[g][:])
                cexp = sm_pool.tile([P, LT, P], bf16, tag="cexp", bufs=1,
                                    name=f"cexp_g{g}_{it}")
                csum = sm_pool.tile([P, LT, GI], f32, tag="csum",
                                    name=f"csum_g{g}_{it}")
                crec = sm_pool.tile([P, LT, GI], f32, tag="crec",
                                    name=f"crec_g{g}_{it}")
                cnx = sm_pool.tile([P, LT, GI, C], bf16, tag="cnx", bufs=4,
                                   name=f"cnx_g{g}_{it}")
                for h in range(2):
                    lts = slice(2 * h, 2 * h + 2)
                    nc.scalar.activation(cexp[:, lts, :], b_tile[g][:, lts, :],
                                         AF.Exp)
                    nc.vector.tensor_reduce(
                        csum[:, lts, :],
                        cexp[:, lts, :].rearrange("p lt (i c) -> p lt i c",
                                                  i=GI),
                        AX.X, ALU.add)
                    nc.vector.reciprocal(crec[:, lts, :], csum[:, lts, :])
                    nc.vector.tensor_tensor(
                        cnx[:, lts, :, :],
                        cexp[:, lts, :].rearrange("p lt (i c) -> p lt i c",
                                                  i=GI),
                        crec[:, lts, :].unsqueeze(3)
                            .broadcast_to((P, 2, GI, C)),
                        ALU.mult,
                    )
                tap("dbg_c1", cnx[:])
                c_cur[g] = cnx
                # (c, i)-ordered copy for the colsum matmul (off the
                # critical chain -- only needed by next iter's colsum)
                cci = sm_pool.tile([P, LT, C, GI], bf16, tag="cci", bufs=2,
                                   name=f"cci_g{g}_{it}")
                nc.vector.tensor_tensor(
                    cci[:],
                    cexp[:].rearrange("p lt (i c) -> p lt c i", i=GI),
                    crec[:].unsqueeze(2).broadcast_to((P, LT, C, GI)),
                    ALU.mult,
                )
                c_ci[g] = cci

            return [phA, phB, phC1, phC2, phD]

        # software-pipelined schedule: group 1 runs one iteration behind
        # group 0 so each group's serial chain is hidden by the other's
        # matmul phases.
        for f in make_phases(0, 0):
            f()
        pa = make_phases(0, 1)
        pb = make_phases(1, 0)
        pa[0](); pa[1](); pb[0](); pa[2](); pb[1]()
        pa[3](); pa[4]()
        for i in range(GI, BPC):
            load_xt(i)
        pb[2](); pb[3](); pb[4]()
        # tail round: slide a's remaining phases between b's to cover
        # b's serial chains (a has only Y+S left on its final iter)
        pa = make_phases(0, 2)
        pb = make_phases(1, 1)
        pa[0](); pb[0](); pb[1](); pa[1](); pb[2](); pb[3](); pb[4]()
        for f in make_phases(1, 2):
            f()

    nc.compile()
    return nc


_NC_CACHE = None


def _get_nc():
    global _NC_CACHE
    if _NC_CACHE is None:
        _NC_CACHE = build_kernel()
    return _NC_CACHE


def _make_consts():
    ident = np.eye(P, dtype=_BF16)
    # yT perm: out col j=4c+i takes y row 32i+c -> permy[32i+c, 4c+i] = 1
    permy = np.zeros((P, P), dtype=_BF16)
    for c in range(C):
        for i in range(GI):
            permy[32 * i + c, 4 * c + i] = 1
    # vT perm: out col 64j+16i+t takes v row 4(2t+j)+i (contiguous vflat runs)
    permv = np.zeros((P, P), dtype=_BF16)
    for t in range(MT):
        for j in range(2):
            for i in range(GI):
                permv[4 * (2 * t + j) + i, 64 * j + 16 * i + t] = 1
    cunif = np.full((P, C), 1.0 / C, dtype=_BF16)
    # biasp[4c+i, o] = fc_b-independent: filled in kernel() (needs fc_b)
    # maskall[p, t, c'] = (c' == 2t + p//64)
    pp = np.arange(P)
    tt = np.arange(MT)
    cc = np.arange(C)
    maskall = (cc[None, None, :] == (2 * tt[None, :, None] + pp[:, None, None] // O)
               ).astype(np.float32).reshape(P, MT * C).astype(_BF16)
    em = (np.arange(8)[None, :] == (pp[:, None] // 4) % 8).astype(np.float32)
    maskbig = np.broadcast_to(
        maskall.reshape(P, MT, 1, C), (P, MT, GI, C)
    ).reshape(P, MT * GI * C).astype(_BF16)
    ones1 = np.ones((P, 1), dtype=_BF16)
    onesl = np.ones((1, 512), dtype=_BF16)
    return ident, permy, permv, cunif, maskbig, em, ones1, onesl


def kernel(inputs, fc_w, fc_b, _trace=False):
    from concourse.bass_utils import run_bass_kernel_spmd

    if _trace:
        _install_ntff_shim()

    nc = _get_nc()

    ident, permy, permv, cunif, maskbig, em, ones1, onesl = _make_consts()

    xf = np.asarray(inputs, dtype=np.float32)
    # x layout [B, 128, LT, D]: x_l[b, lp, lt, d] = x[b, lt*128+lp, d]
    x_l = np.ascontiguousarray(
        xf.reshape(B, LT, P, D).transpose(0, 2, 1, 3)).astype(_BF16)
    # xt layout [B, 128, KD, L]: xt[b, dp, kd, l] = x[b, l, kd*128+dp]
    xt_l = np.ascontiguousarray(
        xf.transpose(0, 2, 1).reshape(B, KD, P, L).transpose(0, 2, 1, 3)
    ).astype(_BF16)
    wf = np.asarray(fc_w, dtype=np.float32)
    # w layout [128, KD, CO]: w[dp, kd, co] = fc_w[kd*128+dp, co]
    w_l = np.ascontiguousarray(
        wf.reshape(KD, P, CO).transpose(1, 0, 2)).astype(_BF16)
    # wt layout [128, MT, D]: wt[p, t, d] = fc_w[d, t*128+p]
    wt_l = np.ascontiguousarray(
        wf.T.reshape(MT, P, D).transpose(1, 0, 2)).astype(_BF16)
    bf = np.asarray(fc_b, dtype=np.float32).reshape(C, O)
    biasp = np.ascontiguousarray(bf[np.arange(P) // 4, :])

    in_maps = []
    for core in range(NCORES):
        in_maps.append({
            "x": x_l[core * BPC:(core + 1) * BPC],
            "xt": xt_l[core * BPC:(core + 1) * BPC],
            "w": w_l,
            "wt": wt_l,
            "biasp": biasp,
            "maskbig": maskbig,
            "em": em,
            "permy": permy,
            "permv": permv,
            "ident": ident,
            "cunif": cunif,
            "ones1": ones1,
            "onesl": onesl,
        })

    res = run_bass_kernel_spmd(
        nc, in_maps, core_ids=list(range(NCORES)), trace=_trace,
    )
    out = np.concatenate(
        [res.results[core]["v"].reshape(NGRP, C, GI, O)
         .transpose(0, 2, 1, 3).reshape(BPC, C, O)
         for core in range(NCORES)],
        axis=0,
    )
    if _trace:
        kernel.last_exec_time_ns = res.exec_time_ns
        kernel.last_results = res
    return out
